# revision 15
# baseline (speedup 1.0000x reference)
"""Trainium2 Bass kernel: 7x7 valid 2D cross-correlation of an 8192x8192
fp32 image plus scalar bias, row-sharded across 8 NeuronCores.

Formulation (per core): the y-direction 7-tap convolution for a fixed kernel
column dx is a banded matmul: out_dx[y, x] = sum_r A_dx[r, y] * X[r, x] with
A_dx[r, y] = K[r - y, dx].  The full conv accumulates the 7 dx terms in PSUM
with the moving operand (image columns) shifted by dx.  Matmuls run in bf16
(inputs bf16, fp32 PSUM accumulate); the banded weight blocks are padded to
128 columns so the compiler's fast-weight-load path engages.

Work distribution: 8186 output rows = 68 bands of <=122 rows.  Each core gets
8 full bands (rows 976*i .. 976*i+976) plus HALF of one of bands 64..67
(8 column tiles), i.e. 136 (band, col-tile) units/core instead of 9 full
bands = 144 — the PE-time quantum is a 512-column matmul pass, so the old
layout wasted 8 units/core on a mostly-empty 9th band.  The half-band is
processed FIRST: its input is only ~1 MB, so the PE starts as soon as the
DMA rings come up instead of waiting for a full 2.1 MB slab.  Output is
stored per 1024-column pair tile immediately after its PSUM drain, so the
kernel tail after the last matmul is one small store, not a 2 MB band store.
"""

import numpy as np
import ml_dtypes

import concourse.bass as bass
import concourse.mybir as mybir
from concourse.tile import TileContext
from concourse.bass_utils import run_bass_kernel_spmd

H = W = 8192
KH = KW = 7
OH = OW = H - KH + 1          # 8186
N_CORES = 8
BAND_IN = 128                 # input rows per matmul band (partition dim)
BAND_OUT = BAND_IN - KH + 1   # 122 output rows per band
APAD = 128                    # A block columns (padded from BAND_OUT for FWL)
COL_TILE = 512                # moving-operand free dim (one PSUM bank, fp32)
F32 = mybir.dt.float32
BF16 = mybir.dt.bfloat16

MAIN_BANDS = 8                # full bands per core
MAIN_OUT = MAIN_BANDS * BAND_OUT      # 976
MAIN_IN = MAIN_OUT + KH - 1           # 982
HALF_TILES = 8                # col tiles in the half band
HALF_OUT_COLS = HALF_TILES * COL_TILE # 4096
HALF_IN_COLS = HALF_OUT_COLS + 8      # 4104 (6-col halo, padded to 8)

# Results object of the most recent hardware run (for test harnesses).
LAST_RESULTS = None


def _split_multi_waits(nc):
    """Walrus in this toolchain accepts at most ONE sync-wait per
    instruction; Tile's scheduler may attach several.  Hoist the extras onto
    single-wait InstEventSemaphore instructions inserted just before, on the
    same engine stream (a sequence of waits = AND of the conditions)."""
    uid = 0
    for fn in nc.m.functions:
        for blk in fn.blocks:
            new_list = []
            for inst in blk.instructions:
                si = getattr(inst, "sync_info", None)
                if si is not None and si.on_wait and len(si.on_wait) > 1:
                    waits = list(si.on_wait)
                    for w in waits[:-1]:
                        ev = mybir.InstEventSemaphore(
                            name=f"wait_split_{uid}",
                            ins=[],
                            outs=[],
                            sync_info=mybir.SyncInfo(on_wait=[w], on_update=[]),
                        )
                        uid += 1
                        ev.engine = inst.engine
                        new_list.append(ev)
                    si.on_wait = [waits[-1]]
                new_list.append(inst)
            blk.instructions[:] = new_list


def _build_nc(bias_val):
    nc = bass.Bass()
    Xm = nc.declare_dram_parameter("Xm", [MAIN_IN, W], BF16, isOutput=False)
    Xh = nc.declare_dram_parameter("Xh", [BAND_IN, HALF_IN_COLS], BF16, isOutput=False)
    A = nc.declare_dram_parameter("A", [BAND_IN, KW * APAD], BF16, isOutput=False)
    Om = nc.declare_dram_parameter("Om", [MAIN_OUT, OW], BF16, isOutput=True)
    Oh = nc.declare_dram_parameter("Oh", [BAND_OUT, HALF_OUT_COLS], BF16, isOutput=True)

    with TileContext(nc) as tc:
        with (
            tc.tile_pool(name="const", bufs=1) as cpool,
            tc.tile_pool(name="hx", bufs=1) as hxpool,
            tc.tile_pool(name="x", bufs=4) as xpool,
            tc.tile_pool(name="o", bufs=3) as opool,
            tc.tile_pool(name="ps", bufs=8, space="PSUM") as pspool,
        ):
            # DMA rings serve strictly in order and each entry's wait gates
            # the ring (head-of-line).  Ring capacity is plentiful (~300 GB/s
            # aggregate burst) so the plan is about ISSUE ORDER: tiny gating
            # loads first, loads kept on the gpsimd ring, stores mostly on the
            # sync/scalar rings where their drain-waits can't block loads.
            # All gating loads ride the gpsimd (SWDGE) ring, which spreads a
            # single DMA's rows across all 16 SDMA engines; an HWDGE DMA
            # serializes ~0.6us/row on one engine and would stall the PE.
            a_tile = cpool.tile([BAND_IN, KW * APAD], BF16)
            nc.gpsimd.dma_start(out=a_tile[:, :], in_=A[:, :])

            # Half-band input split: hx_a gates the 6 opening col tiles,
            # hx_b the 2 closing ones (loaded later, used at the very end).
            hx_a0 = hxpool.tile([BAND_IN, 520], BF16, tag="hxa0")
            hx_a = hxpool.tile([BAND_IN, 2568], BF16, tag="hxa")
            hx_b = hxpool.tile([BAND_IN, 1032], BF16, tag="hxb")
            nc.gpsimd.dma_start(out=hx_a0[:, :], in_=Xh[:, 0:520])
            nc.gpsimd.dma_start(out=hx_a[:, :], in_=Xh[:, 512:3080])

            x_tiles = {}

            def issue_load(bi):
                if bi >= MAIN_BANDS:
                    return
                r0 = bi * BAND_OUT
                xt = xpool.tile([BAND_IN, W], BF16, tag="x")
                nc.gpsimd.dma_start(out=xt[0:64, :], in_=Xm[r0 : r0 + 64, :])
                nc.gpsimd.dma_start(out=xt[64:128, :], in_=Xm[r0 + 64 : r0 + 128, :])
                x_tiles[bi] = xt

            issue_load(0)
            issue_load(1)
            issue_load(2)

            def conv_tile(x_tile, x0, w, o_tile, c0):
                """7 accumulating matmuls into a PSUM bank, drain to o_tile."""
                ps = pspool.tile([APAD, COL_TILE], F32)
                for dx in range(KW):
                    nc.tensor.matmul(
                        ps[:, :w],
                        lhsT=a_tile[:, dx * APAD : (dx + 1) * APAD],
                        rhs=x_tile[:, x0 + dx : x0 + dx + w],
                        start=(dx == 0),
                        stop=(dx == KW - 1),
                    )
                nc.vector.tensor_scalar_add(
                    o_tile[:, c0 : c0 + w], ps[:BAND_OUT, :w], float(bias_val)
                )

            # --- opening 6 half-band col tiles: gated only on A + hx_a
            # (~1 MB), the PE starts early and has ~9us of cover while the
            # first two main bands stream in.
            o_ha = opool.tile([BAND_OUT, 6 * COL_TILE], BF16, tag="oha")
            conv_tile(hx_a0, 0, COL_TILE, o_ha, 0)
            for j in range(1, 6):
                conv_tile(hx_a, (j - 1) * COL_TILE, COL_TILE, o_ha, j * COL_TILE)
            nc.sync.dma_start(out=Oh[0:31, 0:3072], in_=o_ha[0:31, :])
            nc.scalar.dma_start(out=Oh[31:61, 0:3072], in_=o_ha[31:61, :])
            nc.sync.dma_start(out=Oh[61:92, 0:3072], in_=o_ha[61:92, :])
            nc.scalar.dma_start(out=Oh[92:BAND_OUT, 0:3072], in_=o_ha[92:BAND_OUT, :])

            # --- main bands: loads stay on the gpsimd ring; stores weighted
            # onto sync/scalar so their drain-waits never block loads.
            # The gpsimd (SWDGE) ring spreads each DMA's rows across all 16
            # SDMA engines (~250+ GB/s); the sync/scalar HWDGE rings serialize
            # ~0.6us/row (~26 GB/s each).  So gpsimd carries the bulk of the
            # stores too; HWDGE gets one 15-row chunk each per band.
            for bi in range(MAIN_BANDS):
                issue_load(bi + 3)
                if bi == 0:
                    nc.gpsimd.dma_start(out=hx_b[:, :], in_=Xh[:, 3072:HALF_IN_COLS])
                x_tile = x_tiles.pop(bi)
                s = bi * BAND_OUT
                if bi < MAIN_BANDS - 1:
                    o_tile = opool.tile([BAND_OUT, OW], BF16, tag="om")
                    for j in range(16):
                        x0 = j * COL_TILE
                        w = min(COL_TILE, OW - x0)
                        conv_tile(x_tile, x0, w, o_tile, x0)
                    nc.gpsimd.dma_start(out=Om[s : s + 92, :], in_=o_tile[0:92, :])
                    nc.sync.dma_start(out=Om[s + 92 : s + 107, :], in_=o_tile[92:107, :])
                    nc.scalar.dma_start(out=Om[s + 107 : s + BAND_OUT, :], in_=o_tile[107:BAND_OUT, :])
                else:
                    # final band drains into two half-width tiles: the left
                    # half stores while the right half computes, so only ~1 MB
                    # remains to flush after the last matmul.
                    o_l = opool.tile([BAND_OUT, 8 * COL_TILE], BF16, tag="oml")
                    for j in range(8):
                        conv_tile(x_tile, j * COL_TILE, COL_TILE, o_l, j * COL_TILE)
                    nc.gpsimd.dma_start(out=Om[s : s + BAND_OUT, 0:4096], in_=o_l[:, :])
                    o_r = opool.tile([BAND_OUT, OW - 8 * COL_TILE], BF16, tag="omr")
                    for j in range(8, 16):
                        x0 = j * COL_TILE
                        w = min(COL_TILE, OW - x0)
                        conv_tile(x_tile, x0, w, o_r, x0 - 4096)
                    nc.gpsimd.dma_start(out=Om[s : s + BAND_OUT, 4096:OW], in_=o_r[:, :])

            # --- closing two half-band col tiles: input resident since band
            # 0; the final tail is a 0.5 MB store on the fast ring.
            o_hb = opool.tile([BAND_OUT, 2 * COL_TILE], BF16, tag="ohb")
            for j in range(6, HALF_TILES):
                conv_tile(hx_b, (j - 6) * COL_TILE, COL_TILE, o_hb, (j - 6) * COL_TILE)
            nc.gpsimd.dma_start(out=Oh[:, 3072:HALF_OUT_COLS], in_=o_hb[:, :])

    _split_multi_waits(nc)
    return nc


def _make_A(K):
    A = np.zeros((BAND_IN, KW * APAD), np.float32)
    for dx in range(KW):
        for y in range(BAND_OUT):
            A[y : y + KH, dx * APAD + y] = K[:, dx]
    return A.astype(ml_dtypes.bfloat16)


def kernel(X, K, bias, _trace=False):
    global LAST_RESULTS
    X = np.asarray(X, dtype=np.float32)
    K = np.asarray(K, dtype=np.float32)
    bias_val = float(np.asarray(bias).reshape(-1)[0])

    A = _make_A(K)
    Xb = X.astype(ml_dtypes.bfloat16)

    in_maps = []
    for i in range(N_CORES):
        xm = Xb[MAIN_OUT * i : MAIN_OUT * i + MAIN_IN]  # contiguous view
        b = 64 + i // 2
        r0 = BAND_OUT * b
        rows = min(BAND_IN, H - r0)  # band 67 has only 18 real input rows
        xh = np.zeros((BAND_IN, HALF_IN_COLS), ml_dtypes.bfloat16)
        if i % 2 == 0:
            xh[:rows, :] = Xb[r0 : r0 + rows, 0:HALF_IN_COLS]
        else:
            xh[:rows, : W - 4096] = Xb[r0 : r0 + rows, 4096:W]
        in_maps.append({"Xm": xm, "Xh": xh, "A": A})

    nc = _build_nc(bias_val)
    res = run_bass_kernel_spmd(nc, in_maps, core_ids=list(range(N_CORES)), trace=_trace)
    LAST_RESULTS = res

    full = np.empty((OH, OW), np.float32)
    for i in range(N_CORES):
        full[MAIN_OUT * i : MAIN_OUT * (i + 1)] = res.results[i]["Om"].astype(
            np.float32
        )
        b = 64 + i // 2
        r0 = BAND_OUT * b
        nr = min(BAND_OUT, OH - r0)  # band 67: 12 valid rows
        oh = res.results[i]["Oh"].astype(np.float32)
        if i % 2 == 0:
            full[r0 : r0 + nr, 0:4096] = oh[:nr, :4096]
        else:
            full[r0 : r0 + nr, 4096:OW] = oh[:nr, : OW - 4096]
    return full


# revision 17
# speedup vs baseline: 1.0121x; 1.0121x over previous
"""Trainium2 Bass kernel: 7x7 valid 2D cross-correlation of an 8192x8192
fp32 image plus scalar bias, row-sharded across 8 NeuronCores.

Formulation (per core): the y-direction 7-tap convolution for a fixed kernel
column dx is a banded matmul: out_dx[y, x] = sum_r A_dx[r, y] * X[r, x] with
A_dx[r, y] = K[r - y, dx].  The full conv accumulates the 7 dx terms in PSUM
with the moving operand (image columns) shifted by dx.  Matmuls run in bf16
(inputs bf16, fp32 PSUM accumulate); the banded weight blocks are padded to
128 columns so the compiler's fast-weight-load path engages.

Work distribution: 8186 output rows = 68 bands of <=122 rows.  Each core gets
8 full bands (rows 976*i .. 976*i+976) plus HALF of one of bands 64..67
(8 column tiles), i.e. 136 (band, col-tile) units/core instead of 9 full
bands = 144 — the PE-time quantum is a 512-column matmul pass, so the old
layout wasted 8 units/core on a mostly-empty 9th band.  The half-band is
processed FIRST: its input is only ~1 MB, so the PE starts as soon as the
DMA rings come up instead of waiting for a full 2.1 MB slab.  Output is
stored per 1024-column pair tile immediately after its PSUM drain, so the
kernel tail after the last matmul is one small store, not a 2 MB band store.
"""

import numpy as np
import ml_dtypes

import concourse.bass as bass
import concourse.mybir as mybir
from concourse.tile import TileContext
from concourse.bass_utils import run_bass_kernel_spmd

H = W = 8192
KH = KW = 7
OH = OW = H - KH + 1          # 8186
N_CORES = 8
BAND_IN = 128                 # input rows per matmul band (partition dim)
BAND_OUT = BAND_IN - KH + 1   # 122 output rows per band
APAD = 128                    # A block columns (padded from BAND_OUT for FWL)
COL_TILE = 512                # moving-operand free dim (one PSUM bank, fp32)
F32 = mybir.dt.float32
BF16 = mybir.dt.bfloat16

MAIN_BANDS = 8                # full bands per core
MAIN_OUT = MAIN_BANDS * BAND_OUT      # 976
MAIN_IN = MAIN_OUT + KH - 1           # 982
HALF_TILES = 8                # col tiles in the half band
HALF_OUT_COLS = HALF_TILES * COL_TILE # 4096
HALF_IN_COLS = HALF_OUT_COLS + 8      # 4104 (6-col halo, padded to 8)

# Results object of the most recent hardware run (for test harnesses).
LAST_RESULTS = None


def _split_multi_waits(nc):
    """Walrus in this toolchain accepts at most ONE sync-wait per
    instruction; Tile's scheduler may attach several.  Hoist the extras onto
    single-wait InstEventSemaphore instructions inserted just before, on the
    same engine stream (a sequence of waits = AND of the conditions)."""
    uid = 0
    for fn in nc.m.functions:
        for blk in fn.blocks:
            new_list = []
            for inst in blk.instructions:
                si = getattr(inst, "sync_info", None)
                if si is not None and si.on_wait and len(si.on_wait) > 1:
                    waits = list(si.on_wait)
                    for w in waits[:-1]:
                        ev = mybir.InstEventSemaphore(
                            name=f"wait_split_{uid}",
                            ins=[],
                            outs=[],
                            sync_info=mybir.SyncInfo(on_wait=[w], on_update=[]),
                        )
                        uid += 1
                        ev.engine = inst.engine
                        new_list.append(ev)
                    si.on_wait = [waits[-1]]
                new_list.append(inst)
            blk.instructions[:] = new_list


def _build_nc(bias_val):
    nc = bass.Bass()
    Xm = nc.declare_dram_parameter("Xm", [MAIN_IN, W], BF16, isOutput=False)
    Xh = nc.declare_dram_parameter("Xh", [BAND_IN, HALF_IN_COLS], BF16, isOutput=False)
    A = nc.declare_dram_parameter("A", [BAND_IN, KW * APAD], BF16, isOutput=False)
    # Om rows padded to 8192 cols so every DRAM row store starts 16-KB
    # aligned (16372-B-stride rows made every store partial-line/misaligned).
    Om = nc.declare_dram_parameter("Om", [MAIN_OUT, W], BF16, isOutput=True)
    Oh = nc.declare_dram_parameter("Oh", [BAND_OUT, HALF_OUT_COLS], BF16, isOutput=True)

    with TileContext(nc) as tc:
        with (
            tc.tile_pool(name="const", bufs=1) as cpool,
            tc.tile_pool(name="hx", bufs=1) as hxpool,
            tc.tile_pool(name="x", bufs=4) as xpool,
            tc.tile_pool(name="o", bufs=3) as opool,
            tc.tile_pool(name="ps", bufs=8, space="PSUM") as pspool,
        ):
            # DMA rings serve strictly in order and each entry's wait gates
            # the ring (head-of-line).  Ring capacity is plentiful (~300 GB/s
            # aggregate burst) so the plan is about ISSUE ORDER: tiny gating
            # loads first, loads kept on the gpsimd ring, stores mostly on the
            # sync/scalar rings where their drain-waits can't block loads.
            # All gating loads ride the gpsimd (SWDGE) ring, which spreads a
            # single DMA's rows across all 16 SDMA engines; an HWDGE DMA
            # serializes ~0.6us/row on one engine and would stall the PE.
            a_tile = cpool.tile([BAND_IN, KW * APAD], BF16)
            nc.gpsimd.dma_start(out=a_tile[:, :], in_=A[:, :])

            # Half-band input split: hx_a gates the 6 opening col tiles,
            # hx_b the 2 closing ones (loaded later, used at the very end).
            hx_a0 = hxpool.tile([BAND_IN, 520], BF16, tag="hxa0")
            hx_a = hxpool.tile([BAND_IN, 2568], BF16, tag="hxa")
            hx_b = hxpool.tile([BAND_IN, 1032], BF16, tag="hxb")
            nc.gpsimd.dma_start(out=hx_a0[:, :], in_=Xh[:, 0:520])
            nc.gpsimd.dma_start(out=hx_a[:, :], in_=Xh[:, 512:3080])

            x_tiles = {}

            def issue_load(bi):
                if bi >= MAIN_BANDS:
                    return
                r0 = bi * BAND_OUT
                xt = xpool.tile([BAND_IN, W], BF16, tag="x")
                nc.gpsimd.dma_start(out=xt[0:64, :], in_=Xm[r0 : r0 + 64, :])
                nc.gpsimd.dma_start(out=xt[64:128, :], in_=Xm[r0 + 64 : r0 + 128, :])
                x_tiles[bi] = xt

            issue_load(0)
            issue_load(1)
            issue_load(2)

            def conv_tile(x_tile, x0, w, o_tile, c0):
                """7 accumulating matmuls into a PSUM bank, drain to o_tile."""
                ps = pspool.tile([APAD, COL_TILE], F32)
                for dx in range(KW):
                    nc.tensor.matmul(
                        ps[:, :w],
                        lhsT=a_tile[:, dx * APAD : (dx + 1) * APAD],
                        rhs=x_tile[:, x0 + dx : x0 + dx + w],
                        start=(dx == 0),
                        stop=(dx == KW - 1),
                    )
                nc.vector.tensor_scalar_add(
                    o_tile[:, c0 : c0 + w], ps[:BAND_OUT, :w], float(bias_val)
                )

            # --- opening 6 half-band col tiles: gated only on A + hx_a
            # (~1 MB), the PE starts early and has ~9us of cover while the
            # first two main bands stream in.
            o_ha = opool.tile([BAND_OUT, 6 * COL_TILE], BF16, tag="oha")
            conv_tile(hx_a0, 0, COL_TILE, o_ha, 0)
            for j in range(1, 6):
                conv_tile(hx_a, (j - 1) * COL_TILE, COL_TILE, o_ha, j * COL_TILE)
            nc.sync.dma_start(out=Oh[0:31, 0:3072], in_=o_ha[0:31, :])
            nc.scalar.dma_start(out=Oh[31:61, 0:3072], in_=o_ha[31:61, :])
            nc.sync.dma_start(out=Oh[61:92, 0:3072], in_=o_ha[61:92, :])
            nc.scalar.dma_start(out=Oh[92:BAND_OUT, 0:3072], in_=o_ha[92:BAND_OUT, :])

            # --- main bands: loads stay on the gpsimd ring; stores weighted
            # onto sync/scalar so their drain-waits never block loads.
            # The gpsimd (SWDGE) ring spreads each DMA's rows across all 16
            # SDMA engines (~250+ GB/s); the sync/scalar HWDGE rings serialize
            # ~0.6us/row (~26 GB/s each).  So gpsimd carries the bulk of the
            # stores too; HWDGE gets one 15-row chunk each per band.
            for bi in range(MAIN_BANDS):
                issue_load(bi + 3)
                if bi == 0:
                    nc.gpsimd.dma_start(out=hx_b[:, :], in_=Xh[:, 3072:HALF_IN_COLS])
                x_tile = x_tiles.pop(bi)
                s = bi * BAND_OUT
                if bi < MAIN_BANDS - 1:
                    o_tile = opool.tile([BAND_OUT, OW], BF16, tag="om")
                    for j in range(16):
                        x0 = j * COL_TILE
                        w = min(COL_TILE, OW - x0)
                        conv_tile(x_tile, x0, w, o_tile, x0)
                    nc.gpsimd.dma_start(out=Om[s : s + 92, 0:OW], in_=o_tile[0:92, :])
                    nc.sync.dma_start(out=Om[s + 92 : s + 107, 0:OW], in_=o_tile[92:107, :])
                    nc.scalar.dma_start(out=Om[s + 107 : s + BAND_OUT, 0:OW], in_=o_tile[107:BAND_OUT, :])
                else:
                    # final band drains into two half-width tiles: the left
                    # half stores while the right half computes, so only ~1 MB
                    # remains to flush after the last matmul.
                    o_l = opool.tile([BAND_OUT, 8 * COL_TILE], BF16, tag="oml")
                    for j in range(8):
                        conv_tile(x_tile, j * COL_TILE, COL_TILE, o_l, j * COL_TILE)
                    nc.gpsimd.dma_start(out=Om[s : s + BAND_OUT, 0:4096], in_=o_l[:, :])
                    o_r = opool.tile([BAND_OUT, OW - 8 * COL_TILE], BF16, tag="omr")
                    for j in range(8, 16):
                        x0 = j * COL_TILE
                        w = min(COL_TILE, OW - x0)
                        conv_tile(x_tile, x0, w, o_r, x0 - 4096)
                    nc.gpsimd.dma_start(out=Om[s : s + 61, 4096:OW], in_=o_r[0:61, :])
                    nc.gpsimd.dma_start(out=Om[s + 61 : s + BAND_OUT, 4096:OW], in_=o_r[61:BAND_OUT, :])

            # --- closing two half-band col tiles: input resident since band
            # 0; the final tail is a 0.5 MB store on the fast ring.
            o_hb = opool.tile([BAND_OUT, 2 * COL_TILE], BF16, tag="ohb")
            for j in range(6, HALF_TILES):
                conv_tile(hx_b, (j - 6) * COL_TILE, COL_TILE, o_hb, (j - 6) * COL_TILE)
            nc.gpsimd.dma_start(out=Oh[:, 3072:HALF_OUT_COLS], in_=o_hb[:, :])

    _split_multi_waits(nc)
    return nc


def _make_A(K):
    A = np.zeros((BAND_IN, KW * APAD), np.float32)
    for dx in range(KW):
        for y in range(BAND_OUT):
            A[y : y + KH, dx * APAD + y] = K[:, dx]
    return A.astype(ml_dtypes.bfloat16)


def kernel(X, K, bias, _trace=False):
    global LAST_RESULTS
    X = np.asarray(X, dtype=np.float32)
    K = np.asarray(K, dtype=np.float32)
    bias_val = float(np.asarray(bias).reshape(-1)[0])

    A = _make_A(K)
    Xb = X.astype(ml_dtypes.bfloat16)

    in_maps = []
    for i in range(N_CORES):
        xm = Xb[MAIN_OUT * i : MAIN_OUT * i + MAIN_IN]  # contiguous view
        b = 64 + i // 2
        r0 = BAND_OUT * b
        rows = min(BAND_IN, H - r0)  # band 67 has only 18 real input rows
        xh = np.zeros((BAND_IN, HALF_IN_COLS), ml_dtypes.bfloat16)
        if i % 2 == 0:
            xh[:rows, :] = Xb[r0 : r0 + rows, 0:HALF_IN_COLS]
        else:
            xh[:rows, : W - 4096] = Xb[r0 : r0 + rows, 4096:W]
        in_maps.append({"Xm": xm, "Xh": xh, "A": A})

    nc = _build_nc(bias_val)
    res = run_bass_kernel_spmd(nc, in_maps, core_ids=list(range(N_CORES)), trace=_trace)
    LAST_RESULTS = res

    full = np.empty((OH, OW), np.float32)
    for i in range(N_CORES):
        full[MAIN_OUT * i : MAIN_OUT * (i + 1)] = res.results[i]["Om"][:, :OW].astype(
            np.float32
        )
        b = 64 + i // 2
        r0 = BAND_OUT * b
        nr = min(BAND_OUT, OH - r0)  # band 67: 12 valid rows
        oh = res.results[i]["Oh"].astype(np.float32)
        if i % 2 == 0:
            full[r0 : r0 + nr, 0:4096] = oh[:nr, :4096]
        else:
            full[r0 : r0 + nr, 4096:OW] = oh[:nr, : OW - 4096]
    return full


# revision 18
# speedup vs baseline: 1.0426x; 1.0301x over previous
"""Trainium2 Bass kernel: 7x7 valid 2D cross-correlation of an 8192x8192
fp32 image plus scalar bias, row-sharded across 8 NeuronCores.

Formulation (per core): the y-direction 7-tap convolution for a fixed kernel
column dx is a banded matmul: out_dx[y, x] = sum_r A_dx[r, y] * X[r, x] with
A_dx[r, y] = K[r - y, dx].  The full conv accumulates the 7 dx terms in PSUM
with the moving operand (image columns) shifted by dx.  Matmuls run in bf16
(inputs bf16, fp32 PSUM accumulate); the banded weight blocks are padded to
128 columns so the compiler's fast-weight-load path engages.

Work distribution: 8186 output rows = 68 bands of <=122 rows.  Each core gets
8 full bands (rows 976*i .. 976*i+976) plus HALF of one of bands 64..67
(8 column tiles), i.e. 136 (band, col-tile) units/core instead of 9 full
bands = 144 — the PE-time quantum is a 512-column matmul pass, so the old
layout wasted 8 units/core on a mostly-empty 9th band.  The half-band is
processed FIRST: its input is only ~1 MB, so the PE starts as soon as the
DMA rings come up instead of waiting for a full 2.1 MB slab.  Output is
stored per 1024-column pair tile immediately after its PSUM drain, so the
kernel tail after the last matmul is one small store, not a 2 MB band store.
"""

import numpy as np
import ml_dtypes

import concourse.bass as bass
import concourse.mybir as mybir
from concourse.tile import TileContext
from concourse.bass_utils import run_bass_kernel_spmd

H = W = 8192
KH = KW = 7
OH = OW = H - KH + 1          # 8186
N_CORES = 8
BAND_IN = 128                 # input rows per matmul band (partition dim)
BAND_OUT = BAND_IN - KH + 1   # 122 output rows per band
APAD = 128                    # A block columns (padded from BAND_OUT for FWL)
COL_TILE = 512                # moving-operand free dim (one PSUM bank, fp32)
F32 = mybir.dt.float32
BF16 = mybir.dt.bfloat16

MAIN_BANDS = 8                # full bands per core
MAIN_OUT = MAIN_BANDS * BAND_OUT      # 976
MAIN_IN = MAIN_OUT + KH - 1           # 982
HALF_TILES = 8                # col tiles in the half band
HALF_OUT_COLS = HALF_TILES * COL_TILE # 4096
HALF_IN_COLS = HALF_OUT_COLS + 8      # 4104 (6-col halo, padded to 8)

# Results object of the most recent hardware run (for test harnesses).
LAST_RESULTS = None


def _split_multi_waits(nc):
    """Walrus in this toolchain accepts at most ONE sync-wait per
    instruction; Tile's scheduler may attach several.  Hoist the extras onto
    single-wait InstEventSemaphore instructions inserted just before, on the
    same engine stream (a sequence of waits = AND of the conditions)."""
    uid = 0
    for fn in nc.m.functions:
        for blk in fn.blocks:
            new_list = []
            for inst in blk.instructions:
                si = getattr(inst, "sync_info", None)
                if si is not None and si.on_wait and len(si.on_wait) > 1:
                    waits = list(si.on_wait)
                    for w in waits[:-1]:
                        ev = mybir.InstEventSemaphore(
                            name=f"wait_split_{uid}",
                            ins=[],
                            outs=[],
                            sync_info=mybir.SyncInfo(on_wait=[w], on_update=[]),
                        )
                        uid += 1
                        ev.engine = inst.engine
                        new_list.append(ev)
                    si.on_wait = [waits[-1]]
                new_list.append(inst)
            blk.instructions[:] = new_list


def _build_nc(bias_val):
    nc = bass.Bass()
    Xm = nc.declare_dram_parameter("Xm", [MAIN_IN, W], BF16, isOutput=False)
    Xh = nc.declare_dram_parameter("Xh", [BAND_IN, HALF_IN_COLS], BF16, isOutput=False)
    A = nc.declare_dram_parameter("A", [BAND_IN, KW * APAD], BF16, isOutput=False)
    # Om rows padded to 8192 cols so every DRAM row store starts 16-KB
    # aligned (16372-B-stride rows made every store partial-line/misaligned).
    Om = nc.declare_dram_parameter("Om", [MAIN_OUT, W], BF16, isOutput=True)
    Oh = nc.declare_dram_parameter("Oh", [BAND_OUT, HALF_OUT_COLS], BF16, isOutput=True)

    with TileContext(nc) as tc:
        with (
            tc.tile_pool(name="const", bufs=1) as cpool,
            tc.tile_pool(name="hx", bufs=1) as hxpool,
            tc.tile_pool(name="x", bufs=4) as xpool,
            tc.tile_pool(name="o", bufs=3) as opool,
            tc.tile_pool(name="ps", bufs=8, space="PSUM") as pspool,
        ):
            # DMA rings serve strictly in order and each entry's wait gates
            # the ring (head-of-line).  Ring capacity is plentiful (~300 GB/s
            # aggregate burst) so the plan is about ISSUE ORDER: tiny gating
            # loads first, loads kept on the gpsimd ring, stores mostly on the
            # sync/scalar rings where their drain-waits can't block loads.
            # All gating loads ride the gpsimd (SWDGE) ring, which spreads a
            # single DMA's rows across all 16 SDMA engines; an HWDGE DMA
            # serializes ~0.6us/row on one engine and would stall the PE.
            a_tile = cpool.tile([BAND_IN, KW * APAD], BF16)
            nc.gpsimd.dma_start(out=a_tile[:, :], in_=A[:, :])

            # Half-band input split: hx_a gates the 6 opening col tiles,
            # hx_b the 2 closing ones (loaded later, used at the very end).
            hx_a0 = hxpool.tile([BAND_IN, 520], BF16, tag="hxa0")
            hx_a = hxpool.tile([BAND_IN, 2568], BF16, tag="hxa")
            hx_b = hxpool.tile([BAND_IN, 1032], BF16, tag="hxb")
            nc.gpsimd.dma_start(out=hx_a0[:, :], in_=Xh[:, 0:520])
            nc.gpsimd.dma_start(out=hx_a[:, :], in_=Xh[:, 512:3080])

            x_tiles = {}

            def issue_load(bi):
                if bi >= MAIN_BANDS:
                    return
                r0 = bi * BAND_OUT
                xt = xpool.tile([BAND_IN, W], BF16, tag="x")
                nc.gpsimd.dma_start(out=xt[0:64, :], in_=Xm[r0 : r0 + 64, :])
                nc.gpsimd.dma_start(out=xt[64:128, :], in_=Xm[r0 + 64 : r0 + 128, :])
                x_tiles[bi] = xt

            issue_load(0)
            issue_load(1)
            issue_load(2)

            def conv_tile(x_tile, x0, w, o_tile, c0):
                """7 accumulating matmuls into a PSUM bank, drain to o_tile."""
                ps = pspool.tile([APAD, COL_TILE], F32)
                for dx in range(KW):
                    nc.tensor.matmul(
                        ps[:, :w],
                        lhsT=a_tile[:, dx * APAD : (dx + 1) * APAD],
                        rhs=x_tile[:, x0 + dx : x0 + dx + w],
                        start=(dx == 0),
                        stop=(dx == KW - 1),
                    )
                nc.vector.tensor_scalar_add(
                    o_tile[:, c0 : c0 + w], ps[:BAND_OUT, :w], float(bias_val)
                )

            # --- opening 6 half-band col tiles: gated only on A + hx_a
            # (~1 MB), the PE starts early and has ~9us of cover while the
            # first two main bands stream in.
            o_ha = opool.tile([BAND_OUT, 6 * COL_TILE], BF16, tag="oha")
            conv_tile(hx_a0, 0, COL_TILE, o_ha, 0)
            for j in range(1, 6):
                conv_tile(hx_a, (j - 1) * COL_TILE, COL_TILE, o_ha, j * COL_TILE)
            nc.sync.dma_start(out=Oh[0:31, 0:3072], in_=o_ha[0:31, :])
            nc.scalar.dma_start(out=Oh[31:61, 0:3072], in_=o_ha[31:61, :])
            nc.sync.dma_start(out=Oh[61:92, 0:3072], in_=o_ha[61:92, :])
            nc.scalar.dma_start(out=Oh[92:BAND_OUT, 0:3072], in_=o_ha[92:BAND_OUT, :])

            # --- main bands: loads stay on the gpsimd ring; stores weighted
            # onto sync/scalar so their drain-waits never block loads.
            # The gpsimd (SWDGE) ring spreads each DMA's rows across all 16
            # SDMA engines (~250+ GB/s); the sync/scalar HWDGE rings serialize
            # ~0.6us/row (~26 GB/s each).  So gpsimd carries the bulk of the
            # stores too; HWDGE gets one 15-row chunk each per band.
            for bi in range(MAIN_BANDS):
                issue_load(bi + 3)
                if bi == 0:
                    nc.gpsimd.dma_start(out=hx_b[:, :], in_=Xh[:, 3072:HALF_IN_COLS])
                x_tile = x_tiles.pop(bi)
                s = bi * BAND_OUT
                if bi < MAIN_BANDS - 1:
                    o_tile = opool.tile([BAND_OUT, OW], BF16, tag="om")
                    for j in range(16):
                        x0 = j * COL_TILE
                        w = min(COL_TILE, OW - x0)
                        conv_tile(x_tile, x0, w, o_tile, x0)
                    nc.gpsimd.dma_start(out=Om[s : s + 92, 0:OW], in_=o_tile[0:92, :])
                    nc.sync.dma_start(out=Om[s + 92 : s + 107, 0:OW], in_=o_tile[92:107, :])
                    nc.scalar.dma_start(out=Om[s + 107 : s + BAND_OUT, 0:OW], in_=o_tile[107:BAND_OUT, :])
                else:
                    # final band drains into four quarter-width tiles: each
                    # quarter stores while the next computes, so only ~0.5 MB
                    # remains to flush (spread over all 3 rings) at the end.
                    for q in range(4):
                        c0 = q * 4 * COL_TILE
                        wq = min(4 * COL_TILE, OW - c0)
                        o_q = opool.tile([BAND_OUT, 4 * COL_TILE], BF16, tag="omq")
                        for j in range(4 * q, 4 * q + 4):
                            x0 = j * COL_TILE
                            w = min(COL_TILE, OW - x0)
                            conv_tile(x_tile, x0, w, o_q, x0 - c0)
                        if q < 3:
                            nc.gpsimd.dma_start(
                                out=Om[s : s + 92, c0 : c0 + wq], in_=o_q[0:92, :wq]
                            )
                            eng = (nc.sync, nc.scalar)[q % 2]
                            eng.dma_start(
                                out=Om[s + 92 : s + BAND_OUT, c0 : c0 + wq],
                                in_=o_q[92:BAND_OUT, :wq],
                            )
                        else:
                            nc.gpsimd.dma_start(
                                out=Om[s : s + 61, c0 : c0 + wq], in_=o_q[0:61, :wq]
                            )
                            nc.sync.dma_start(
                                out=Om[s + 61 : s + 92, c0 : c0 + wq], in_=o_q[61:92, :wq]
                            )
                            nc.scalar.dma_start(
                                out=Om[s + 92 : s + BAND_OUT, c0 : c0 + wq],
                                in_=o_q[92:BAND_OUT, :wq],
                            )

            # --- closing two half-band col tiles: input resident since band
            # 0; the final tail is a 0.5 MB store on the fast ring.
            o_hb = opool.tile([BAND_OUT, 2 * COL_TILE], BF16, tag="ohb")
            for j in range(6, HALF_TILES):
                conv_tile(hx_b, (j - 6) * COL_TILE, COL_TILE, o_hb, (j - 6) * COL_TILE)
            nc.gpsimd.dma_start(out=Oh[0:61, 3072:HALF_OUT_COLS], in_=o_hb[0:61, :])
            nc.sync.dma_start(out=Oh[61:92, 3072:HALF_OUT_COLS], in_=o_hb[61:92, :])
            nc.scalar.dma_start(out=Oh[92:BAND_OUT, 3072:HALF_OUT_COLS], in_=o_hb[92:BAND_OUT, :])

    _split_multi_waits(nc)
    return nc


def _make_A(K):
    A = np.zeros((BAND_IN, KW * APAD), np.float32)
    for dx in range(KW):
        for y in range(BAND_OUT):
            A[y : y + KH, dx * APAD + y] = K[:, dx]
    return A.astype(ml_dtypes.bfloat16)


def kernel(X, K, bias, _trace=False):
    global LAST_RESULTS
    X = np.asarray(X, dtype=np.float32)
    K = np.asarray(K, dtype=np.float32)
    bias_val = float(np.asarray(bias).reshape(-1)[0])

    A = _make_A(K)
    Xb = X.astype(ml_dtypes.bfloat16)

    in_maps = []
    for i in range(N_CORES):
        xm = Xb[MAIN_OUT * i : MAIN_OUT * i + MAIN_IN]  # contiguous view
        b = 64 + i // 2
        r0 = BAND_OUT * b
        rows = min(BAND_IN, H - r0)  # band 67 has only 18 real input rows
        xh = np.zeros((BAND_IN, HALF_IN_COLS), ml_dtypes.bfloat16)
        if i % 2 == 0:
            xh[:rows, :] = Xb[r0 : r0 + rows, 0:HALF_IN_COLS]
        else:
            xh[:rows, : W - 4096] = Xb[r0 : r0 + rows, 4096:W]
        in_maps.append({"Xm": xm, "Xh": xh, "A": A})

    nc = _build_nc(bias_val)
    res = run_bass_kernel_spmd(nc, in_maps, core_ids=list(range(N_CORES)), trace=_trace)
    LAST_RESULTS = res

    full = np.empty((OH, OW), np.float32)
    for i in range(N_CORES):
        full[MAIN_OUT * i : MAIN_OUT * (i + 1)] = res.results[i]["Om"][:, :OW].astype(
            np.float32
        )
        b = 64 + i // 2
        r0 = BAND_OUT * b
        nr = min(BAND_OUT, OH - r0)  # band 67: 12 valid rows
        oh = res.results[i]["Oh"].astype(np.float32)
        if i % 2 == 0:
            full[r0 : r0 + nr, 0:4096] = oh[:nr, :4096]
        else:
            full[r0 : r0 + nr, 4096:OW] = oh[:nr, : OW - 4096]
    return full
